# revision 5
# baseline (speedup 1.0000x reference)
"""AttFKANBlock Trainium2 Bass kernel (8 NeuronCores, data-parallel over batch).

Pipeline per batch (4096 tokens, dim=128):
  LN1 (token-major, bn_stats + Newton rsqrt + fused apply)
  -> PE transpose to dim-major
  -> FKAN1: FRAC_SCALE custom DVE op (range reduction, folds LN gamma/beta)
            + ACT Sin -> bf16 features -> PE matmul (2048-dim contraction)
  -> LN2 (dim-major, gpsimd partition reduces + broadcasts)
  -> FKAN2
  -> CBAM channel+spatial attention (the torch .view maps channel c'=token//32,
     spatial l' = 128*(token%32)+dim)
  -> residual add in token-major after PE transpose back -> DMA out.
"""
import numpy as np
import ml_dtypes

import concourse.bass as bass
import concourse.bacc as bacc
import concourse.mybir as mybir
import concourse.tile as tile
from concourse import bass_isa
from concourse.bass_utils import run_bass_kernel_spmd

# ---------------------------------------------------------------- custom DVE ops
# FRAC0:   out = u - round_ne(u), u = in0*s0         (4 stages, 2x_2p capable)
# WRAP_HI: out = y - (y > s1),    y = in0 + s0       (3 stages, 2x_2p capable)
from concourse.dve_ops import DveOp, OPS, CUSTOM_DVE_SPECS, _SUB_OPCODE_FOR_NAME
import concourse.dve_ops as _dve_ops_mod
from concourse.dve_spec import Spec, Src0, C0, C1, lower as _dve_lower
from concourse.dve_uop import (DveOpSpec, UopConfig, UopDpConfig, InpSel, OutSel,
                               OutPath, AluInp, DelayInp, AluOp as UAluOp, Trigger,
                               ENABLE, DISABLE)

_MAGIC = 12582912.0  # 1.5 * 2**23


def _ref_frac0(in0, in1, s0, s1, imm2):
    u = np.float32(in0.astype(np.float32) * np.float32(s0))
    v = np.float32(u + np.float32(s1))
    r = np.float32(v - np.float32(s1))
    return np.float32(u - r)


def _ref_wrap_hi(in0, in1, s0, s1, imm2):
    y = np.float32(in0.astype(np.float32) + np.float32(s0))
    return np.float32(y - (y > np.float32(s1)).astype(np.float32))


def _dp(op=UAluOp.BYPASS, a0=AluInp.PREV_ALU_OUT, a1=AluInp.PREV_ALU_OUT,
        dly=None, den=None):
    d = [DelayInp.PREV_DELAY] * 7
    e = [0] * 7
    if dly:
        for k, v in dly.items():
            d[k] = v
    if den:
        for k in den:
            e[k] = 1
    return UopDpConfig(op=op, alu_src0=a0, alu_src1=a1, delay=d,
                       alu_out_enable=1, delay_enable=e)


def _mk_uop_2x2p(blocks):
    u = UopConfig()
    u.inp = [InpSel.ZERO, InpSel.SRC_0, InpSel.CONST_0, InpSel.CONST_1,
             InpSel.SRC_1, InpSel.ZERO, InpSel.ZERO, InpSel.ZERO]
    u.inp_enable = [0, 1, 1, 1, 1, 0, 0, 0]
    u.out = {OutPath.WR0_LO: OutSel.DELAY_0, OutPath.WR0_HI: OutSel.ALU_OUT,
             OutPath.WR1_LO: OutSel.ALU_OUT, OutPath.WR1_HI: OutSel.ALU_OUT}
    u.out_enable = {OutPath.WR0_LO: 1, OutPath.WR0_HI: 1,
                    OutPath.WR1_LO: 0, OutPath.WR1_HI: 0}
    u.trigger = (Trigger.SRC_TENSOR_DONE, Trigger.NONE, Trigger.NONE)
    u.datapath_config = blocks
    return u


PA = AluInp.PREV_ALU_OUT
D0, D1, D2, D3 = (AluInp.PREV_DELAY_0, AluInp.PREV_DELAY_1,
                  AluInp.PREV_DELAY_2, AluInp.PREV_DELAY_3)
DP_PASS = DelayInp.PREV_DELAY
DP_ALU = DelayInp.PREV_ALU_OUT
DP_CUR = DelayInp.CURR_ALU_OUT

# FRAC0 2x_2p: chain A on b0-b3 (in SRC_0 -> WR0 via delay0), chain B on b4-b7
_FRAC0_2X2P = _mk_uop_2x2p([
    # b0: uA = srcA * C0 ; pass C0(d1) C1(d2) srcB(d3)
    _dp(UAluOp.MULTIPLY, D0, D1, den=[1, 2, 3]),
    # b1: uA + C1 ; capture uA on d0; pass others
    _dp(UAluOp.ADD, PA, D2, dly={0: DP_ALU}, den=[0, 1, 2, 3]),
    # b2: rA = (uA+C1) - C1 ; pass uA(d0) C0(d1) C1(d2) srcB(d3)
    _dp(UAluOp.SUBTRACT, PA, D2, den=[0, 1, 2, 3]),
    # b3: outA = uA - rA ; capture outA on d0; pass C0 C1 srcB
    _dp(UAluOp.SUBTRACT, D0, PA, dly={0: DP_CUR}, den=[0, 1, 2, 3]),
    # b4: uB = srcB * C0 ; pass outA(d0) C1(d2)
    _dp(UAluOp.MULTIPLY, D3, D1, den=[0, 2]),
    # b5: uB + C1 ; capture uB on d1; pass outA(d0) C1(d2)
    _dp(UAluOp.ADD, PA, D2, dly={1: DP_ALU}, den=[0, 1, 2]),
    # b6: rB ; pass outA(d0) uB(d1)
    _dp(UAluOp.SUBTRACT, PA, D2, den=[0, 1]),
    # b7: outB = uB - rB ; pass outA(d0)
    _dp(UAluOp.SUBTRACT, D1, PA, den=[0]),
])

# WRAP_HI 2x_2p: chain A b0-b2, chain B b3-b5, bypass b6-b7
_WRAP_2X2P = _mk_uop_2x2p([
    # b0: yA = srcA + C0 ; pass C0(d1) C1(d2) srcB(d3)
    _dp(UAluOp.ADD, D0, D1, den=[1, 2, 3]),
    # b1: gA = yA > C1 ; capture yA on d0
    _dp(UAluOp.IS_GT, PA, D2, dly={0: DP_ALU}, den=[0, 1, 2, 3]),
    # b2: outA = yA - gA ; capture outA on d0
    _dp(UAluOp.SUBTRACT, D0, PA, dly={0: DP_CUR}, den=[0, 1, 2, 3]),
    # b3: yB = srcB + C0 ; pass outA(d0) C1(d2)
    _dp(UAluOp.ADD, D3, D1, den=[0, 2]),
    # b4: gB = yB > C1 ; capture yB on d1
    _dp(UAluOp.IS_GT, PA, D2, dly={1: DP_ALU}, den=[0, 1, 2]),
    # b5: outB = yB - gB ; pass outA(d0)
    _dp(UAluOp.SUBTRACT, D1, PA, den=[0]),
    # b6, b7: bypass outB down the ALU path, outA down d0
    _dp(UAluOp.BYPASS, PA, PA, den=[0]),
    _dp(UAluOp.BYPASS, PA, PA, den=[0]),
])


def _register_op(name, spec, uops_2x2p):
    if name in _SUB_OPCODE_FOR_NAME:
        return next(op for op in OPS if op.name == name)
    row = max(_SUB_OPCODE_FOR_NAME.values()) + 1
    assert row < 0x20
    _SUB_OPCODE_FOR_NAME[name] = row
    shas = {}
    specs = {}
    for ver in ("v3", "v4"):
        u1 = _dve_lower(spec, ver=ver)
        import copy as _copy
        ds = DveOpSpec(name=name, opcode=row, uops=u1,
                       uops_2x=[_copy.deepcopy(x) for x in u1],
                       uops_2x_2p=[uops_2x2p] if ver == "v3" else None,
                       uops_4x=None,
                       perf_max=2 if ver == "v3" else 0,
                       rd1_en=False)
        if ver != "v3":
            ds = DveOpSpec(name=name, opcode=row, uops=u1, rd1_en=False)
        shas[ver] = ds.sha(ver)
        specs[ver] = ds
    op = DveOp(name, spec, subdim=False, uops_sha=shas)
    OPS.append(op)
    CUSTOM_DVE_SPECS[name] = spec
    for ver in ("v3", "v4"):
        _dve_ops_mod._COMPILE_CACHE[(name, ver)] = specs[ver]
    return op


_u0 = Src0 * C0
FRAC0 = _register_op("FRAC0_ANT",
                     Spec(body=_u0 - ((_u0 + C1) - C1), reference=_ref_frac0),
                     _FRAC0_2X2P)
_y0 = Src0 + C0
WRAP_HI = _register_op("WRAP_HI_ANT",
                       Spec(body=_y0 - (_y0 > C1), reference=_ref_wrap_hi),
                       _WRAP_2X2P)

from concourse.dve_spec import C2 as _C2


def _ref_frac_ph(in0, in1, s0, s1, imm2):
    u = np.float32(in0.astype(np.float32) * np.float32(s0) + np.float32(s1))
    v = np.float32(u + np.float32(imm2))
    r = np.float32(v - np.float32(imm2))
    return np.float32(u - r)


def _register_plain(name, spec):
    if name in _SUB_OPCODE_FOR_NAME:
        return next(op for op in OPS if op.name == name)
    row = max(_SUB_OPCODE_FOR_NAME.values()) + 1
    assert row < 0x20
    _SUB_OPCODE_FOR_NAME[name] = row
    shas = {}
    for ver in ("v3", "v4"):
        ds = DveOpSpec(name=name, opcode=row, uops=_dve_lower(spec, ver=ver),
                       rd1_en=False)
        shas[ver] = ds.sha(ver)
    op = DveOp(name, spec, subdim=False, uops_sha=shas)
    OPS.append(op)
    CUSTOM_DVE_SPECS[name] = spec
    return op


_uph = Src0 * C0 + C1
FRAC_PH = _register_plain("FRAC_PH_ANT",
                          Spec(body=_uph - ((_uph + _C2) - _C2),
                               reference=_ref_frac_ph))


def _frac_ph(nc, out, in_, s0, s1):
    return nc.vector._custom_dve(FRAC_PH, out=out, in0=in_, s0=s0, s1=s1,
                                 imm2=_MAGIC)


# ---- harmonic-ladder ops (2-input, f16, 2x_1p packed: 2 tokens/cycle) ----
# LMAD: out = (in0*s0)*in1 - s1   (c2k = 2*ck*ck - 1, s2k = 2*ck*sk)
# LODD: out = (in0*s0 + s1)*in1   (c3 = (2c2-1)*c1, s3 = (2c2+1)*s1)
from concourse.dve_spec import Src1 as _Src1


def _ref_lmad(in0, in1, s0, s1, imm2):
    return np.float32(np.float32(in0.astype(np.float32) * np.float32(s0))
                      * in1.astype(np.float32) - np.float32(s1))


def _ref_lodd(in0, in1, s0, s1, imm2):
    return np.float32(np.float32(in0.astype(np.float32) * np.float32(s0)
                                 + np.float32(s1)) * in1.astype(np.float32))


def _mk_uop_tt2x(blocks):
    """TT-shaped 2x_1p uop: lanes a=SRC_0, b=SRC_1, C0, C1, a'=SRC_0_HI,
    b'=SRC_1_HI; chain A on b0-b2 -> delay0, chain B on b3-b5 -> ALU out."""
    u = UopConfig()
    u.inp = [InpSel.ZERO, InpSel.SRC_0, InpSel.SRC_1, InpSel.CONST_0,
             InpSel.CONST_1, InpSel.SRC_0_HI, InpSel.SRC_1_HI, InpSel.ZERO]
    u.inp_enable = [0, 1, 1, 1, 1, 1, 1, 0]
    u.out = {OutPath.WR0_LO: OutSel.DELAY_0, OutPath.WR0_HI: OutSel.ALU_OUT,
             OutPath.WR1_LO: OutSel.ALU_OUT, OutPath.WR1_HI: OutSel.ALU_OUT}
    u.out_enable = {OutPath.WR0_LO: 1, OutPath.WR0_HI: 1,
                    OutPath.WR1_LO: 0, OutPath.WR1_HI: 0}
    u.trigger = (Trigger.SRC_TENSOR_DONE, Trigger.NONE, Trigger.NONE)
    u.datapath_config = blocks
    return u


D4, D5 = AluInp.PREV_DELAY_4, AluInp.PREV_DELAY_5

# lane map at block0: D0=a, D1=b, D2=C0, D3=C1, D4=a_hi, D5=b_hi
_LMAD_2X = _mk_uop_tt2x([
    _dp(UAluOp.MULTIPLY, D0, D2, den=[1, 2, 3, 4, 5]),      # tA = a*C0
    _dp(UAluOp.MULTIPLY, PA, D1, den=[2, 3, 4, 5]),          # tA *= b
    _dp(UAluOp.SUBTRACT, PA, D3, dly={0: DP_CUR},            # outA = tA - C1
        den=[0, 2, 3, 4, 5]),
    _dp(UAluOp.MULTIPLY, D4, D2, den=[0, 3, 5]),             # tB = a'*C0
    _dp(UAluOp.MULTIPLY, PA, D5, den=[0, 3]),                # tB *= b'
    _dp(UAluOp.SUBTRACT, PA, D3, den=[0]),                   # outB = tB - C1
    _dp(UAluOp.BYPASS, PA, PA, den=[0]),
    _dp(UAluOp.BYPASS, PA, PA, den=[0]),
])

_LODD_2X = _mk_uop_tt2x([
    _dp(UAluOp.MULTIPLY, D0, D2, den=[1, 2, 3, 4, 5]),      # tA = a*C0
    _dp(UAluOp.ADD, PA, D3, den=[1, 2, 3, 4, 5]),            # tA += C1
    _dp(UAluOp.MULTIPLY, PA, D1, dly={0: DP_CUR},            # outA = tA*b
        den=[0, 2, 3, 4, 5]),
    _dp(UAluOp.MULTIPLY, D4, D2, den=[0, 3, 5]),             # tB = a'*C0
    _dp(UAluOp.ADD, PA, D3, den=[0, 5]),                     # tB += C1
    _dp(UAluOp.MULTIPLY, PA, D5, den=[0]),                   # outB = tB*b'
    _dp(UAluOp.BYPASS, PA, PA, den=[0]),
    _dp(UAluOp.BYPASS, PA, PA, den=[0]),
])


def _register_tt_op(name, spec, uops_2x1p):
    if name in _SUB_OPCODE_FOR_NAME:
        return next(op for op in OPS if op.name == name)
    row = max(_SUB_OPCODE_FOR_NAME.values()) + 1
    assert row < 0x20
    _SUB_OPCODE_FOR_NAME[name] = row
    shas = {}
    specs = {}
    for ver in ("v3", "v4"):
        u1 = _dve_lower(spec, ver=ver)
        if ver == "v3":
            assert len(u1) == 1, f"{name}: expected single-uop lowering"
            ds = DveOpSpec(name=name, opcode=row, uops=u1,
                           uops_2x=[uops_2x1p], uops_2x_2p=None, uops_4x=None,
                           perf_max=1, rd1_en=True)
        else:
            ds = DveOpSpec(name=name, opcode=row, uops=u1, rd1_en=True)
        shas[ver] = ds.sha(ver)
        specs[ver] = ds
    op = DveOp(name, spec, subdim=False, uops_sha=shas)
    OPS.append(op)
    CUSTOM_DVE_SPECS[name] = spec
    for ver in ("v3", "v4"):
        _dve_ops_mod._COMPILE_CACHE[(name, ver)] = specs[ver]
    return op


LMAD = _register_tt_op("LMAD_ANT",
                       Spec(body=(Src0 * C0) * _Src1 - C1, reference=_ref_lmad),
                       _LMAD_2X)
LODD = _register_tt_op("LODD_ANT",
                       Spec(body=(Src0 * C0 + C1) * _Src1, reference=_ref_lodd),
                       _LODD_2X)


def _lmad(nc, out, a, b, s0, s1):
    return nc.vector._custom_dve(LMAD, out=out, in0=a, in1=b, s0=s0, s1=s1)


def _lodd(nc, out, a, b, s0, s1):
    return nc.vector._custom_dve(LODD, out=out, in0=a, in1=b, s0=s0, s1=s1)


def _frac0(nc, out, in_, s0):
    return nc.vector._custom_dve(FRAC0, out=out, in0=in_, s0=s0, s1=_MAGIC)


def _wrap_hi(nc, out, in_, s0):
    return nc.vector._custom_dve(WRAP_HI, out=out, in0=in_, s0=s0, s1=0.5)


# ---------------------------------------------------------------- constants
B, L, D, G = 16, 4096, 128, 8
RED = 8          # D // 16
NF = 2 * G       # 16 features per input dim (cos/sin x 8 harmonics)
NCORES = 8
BPC = B // NCORES          # 2 batches per core
TOK = BPC * L              # 8192 tokens per core
PI = float(np.pi)
EPS = 1e-5
NT = L // 128              # 32 token tiles per batch
A = mybir.AluOpType
F32, BF16, F16 = mybir.dt.float32, mybir.dt.bfloat16, mybir.dt.float16
AF = mybir.ActivationFunctionType


def _newton_rsqrt(nc, pool, var_ap, p, n, tag):
    """rsqrt(var + EPS) on a [p, n] f32 tile chain. Returns R tile [p, n]."""
    vp = pool.tile([p, n], F32, tag=f"{tag}_v")
    nc.vector.tensor_scalar_add(out=vp[:, :], in0=var_ap, scalar1=EPS)
    y = pool.tile([p, n], F32, tag=f"{tag}_y")
    nc.vector.tensor_scalar(out=y[:, :], in0=vp[:, :], scalar1=-0.5, scalar2=1.5,
                            op0=A.mult, op1=A.add)
    nc.vector.tensor_scalar_max(out=y[:, :], in0=y[:, :], scalar1=0.19)
    a_t = pool.tile([p, n], F32, tag=f"{tag}_a")
    c_t = pool.tile([p, n], F32, tag=f"{tag}_c")
    for _ in range(6):
        nc.vector.tensor_tensor(out=a_t[:, :], in0=y[:, :], in1=y[:, :], op=A.mult)
        nc.vector.scalar_tensor_tensor(out=c_t[:, :], in0=vp[:, :], scalar=-0.5,
                                       in1=a_t[:, :], op0=A.mult, op1=A.mult)
        nc.vector.scalar_tensor_tensor(out=y[:, :], in0=c_t[:, :], scalar=1.5,
                                       in1=y[:, :], op0=A.add, op1=A.mult)
    return y


def build_program(reps=1):
    nc = bacc.Bacc("TRN2", target_bir_lowering=False, debug=False, num_devices=NCORES,
                   enable_asserts=False)
    x_d = nc.dram_tensor("x", [TOK, D], F32, kind="ExternalInput")
    w1_d = nc.dram_tensor("w1f", [NF, D, D], F16, kind="ExternalInput")
    w2_d = nc.dram_tensor("w2f", [NF, D, D], F16, kind="ExternalInput")
    sc1_d = nc.dram_tensor("sc1", [D, NF], F32, kind="ExternalInput")
    sb1_d = nc.dram_tensor("sb1", [D, NF], F32, kind="ExternalInput")
    sc2_d = nc.dram_tensor("sc2", [D, NF], F32, kind="ExternalInput")
    sb2_d = nc.dram_tensor("sb2", [D, NF], F32, kind="ExternalInput")
    b1_d = nc.dram_tensor("fb1", [D, 1], F32, kind="ExternalInput")
    b2_d = nc.dram_tensor("fb2", [D, 1], F32, kind="ExternalInput")
    w1t_d = nc.dram_tensor("w1t", [D, RED], F32, kind="ExternalInput")
    w2t_d = nc.dram_tensor("w2t", [RED, D], F32, kind="ExternalInput")
    cw_d = nc.dram_tensor("cw", [1, 14], F32, kind="ExternalInput")
    out_d = nc.dram_tensor("out", [TOK, D], F32, kind="ExternalOutput")
    rb_d = nc.dram_tensor("rbounce", [BPC, L], F32)
    cab_d = nc.dram_tensor("cabounce", [BPC, D], F32)

    from contextlib import ExitStack
    from concourse.masks import make_identity

    with tile.TileContext(nc) as tc, ExitStack() as ctx:
        singles = ctx.enter_context(tc.tile_pool(name="singles", bufs=1))
        xpool = ctx.enter_context(tc.tile_pool(name="xtok", bufs=2))
        big = ctx.enter_context(tc.tile_pool(name="big", bufs=4))
        mpool = ctx.enter_context(tc.tile_pool(name="mtile", bufs=3))
        fpool = ctx.enter_context(tc.tile_pool(name="ftile", bufs=1))
        small = ctx.enter_context(tc.tile_pool(name="small", bufs=2))
        rpool = ctx.enter_context(tc.tile_pool(name="rrow", bufs=1))
        xnorm = ctx.enter_context(tc.tile_pool(name="xnorm", bufs=6))
        otok = ctx.enter_context(tc.tile_pool(name="otok", bufs=3))
        mmps = ctx.enter_context(tc.tile_pool(name="mmps", bufs=2, space="PSUM"))
        tpps = mmps
        typs = mmps

        # ---- constants / weights resident in SBUF
        W1s = singles.tile([D, NF, D], F16)
        nc.sync.dma_start(out=W1s[:, :, :], in_=w1_d.ap().rearrange("f i o -> i f o"))
        W2s = singles.tile([D, NF, D], F16)
        nc.sync.dma_start(out=W2s[:, :, :], in_=w2_d.ap().rearrange("f i o -> i f o"))
        SC1 = singles.tile([D, NF], F32)
        nc.sync.dma_start(out=SC1[:, :], in_=sc1_d[:, :])
        SB1 = singles.tile([D, NF], F32)
        nc.sync.dma_start(out=SB1[:, :], in_=sb1_d[:, :])
        SC2 = singles.tile([D, NF], F32)
        nc.sync.dma_start(out=SC2[:, :], in_=sc2_d[:, :])
        SB2 = singles.tile([D, NF], F32)
        nc.sync.dma_start(out=SB2[:, :], in_=sb2_d[:, :])
        B1c = singles.tile([D, 1], F32)
        nc.sync.dma_start(out=B1c[:, :], in_=b1_d[:, :])
        B2c = singles.tile([D, 1], F32)
        nc.sync.dma_start(out=B2c[:, :], in_=b2_d[:, :])
        W1T = singles.tile([D, RED], F32)
        nc.sync.dma_start(out=W1T[:, :], in_=w1t_d[:, :])
        W2T = singles.tile([RED, D], F32)
        nc.sync.dma_start(out=W2T[:, :], in_=w2t_d[:, :])
        CW = singles.tile([32, 14], F32)
        nc.sync.dma_start(out=CW[:, :], in_=bass.AP(tensor=cw_d, offset=0,
                                                    ap=[[0, 32], [1, 14]]))
        IDN = singles.tile([D, D], F32)
        make_identity(nc, IDN[:, :])
        ONESC = singles.tile([D, 1], F32)
        nc.vector.memset(ONESC[:, :], 1.0)

        x_r = x_d.ap().rearrange("(a p) d -> p a d", p=128)      # [128, 64, 128]
        out_r = out_d.ap().rearrange("(a p) d -> p a d", p=128)  # [128, 64, 128]

        def fkan(XN, SC, SB, Ws, bias_col, relu, Yout):
            """XN (128 dims x 4096 tok f32) -> Yout (128 out x 4096 tok f32).

            Bases k=1,5,7 via FRAC+ACT Sin; harmonics k=2,3,4,6,8 via the
            f16 DVE ladder (LMAD/LODD at 2 tok/cycle)."""
            for half in range(2):
                cs = slice(2048 * half, 2048 * (half + 1))
                ps = mmps.tile([128, 2048], F32, tag="mm")
                P = {}
                for k in (1, 5, 7):
                    fs, fc = G + (k - 1), (k - 1)
                    fb = mpool.tile([128, 2048], F16, tag="m")
                    _frac0(nc, fb[:, :], XN[:, cs], SC[:, fs:fs + 1])
                    sk = fpool.tile([128, 2048], F16, tag=f"s{k}")
                    nc.scalar.activation(sk[:, :], fb[:, :], AF.Sin,
                                         bias=SB[:, fs:fs + 1], scale=2 * PI)
                    P[('s', k)] = sk
                    fb2 = mpool.tile([128, 2048], F16, tag="m")
                    _frac_ph(nc, fb2[:, :], XN[:, cs], SC[:, fc:fc + 1],
                             SB[:, fc:fc + 1])
                    ck = fpool.tile([128, 2048], F16, tag=f"c{k}")
                    nc.scalar.activation(ck[:, :], fb2[:, :], AF.Sin,
                                         bias=0.0, scale=2 * PI)
                    P[('c', k)] = ck
                # ladder: (src_a, src_b, op, s0, s1)
                for (t, k), (a, b, op, s0, s1) in (
                        (('c', 2), (('c', 1), ('c', 1), _lmad, 2.0, 1.0)),
                        (('s', 2), (('c', 1), ('s', 1), _lmad, 2.0, 0.0)),
                        (('c', 3), (('c', 2), ('c', 1), _lodd, 2.0, -1.0)),
                        (('s', 3), (('c', 2), ('s', 1), _lodd, 2.0, 1.0)),
                        (('c', 4), (('c', 2), ('c', 2), _lmad, 2.0, 1.0)),
                        (('s', 4), (('c', 2), ('s', 2), _lmad, 2.0, 0.0)),
                        (('c', 6), (('c', 3), ('c', 3), _lmad, 2.0, 1.0)),
                        (('s', 6), (('c', 3), ('s', 3), _lmad, 2.0, 0.0)),
                        (('c', 8), (('c', 4), ('c', 4), _lmad, 2.0, 1.0)),
                        (('s', 8), (('c', 4), ('s', 4), _lmad, 2.0, 0.0))):
                    pk = fpool.tile([128, 2048], F16, tag=f"{t}{k}")
                    op(nc, pk[:, :], P[a][:, :], P[b][:, :], s0, s1)
                    P[(t, k)] = pk
                # matmuls in plane-readiness order; accumulate into ps
                order = [('s', 1), ('c', 1), ('s', 5), ('c', 5), ('s', 7),
                         ('c', 7), ('c', 2), ('s', 2), ('c', 3), ('s', 3),
                         ('c', 4), ('s', 4), ('c', 6), ('s', 6), ('c', 8),
                         ('s', 8)]
                for mi, (t, k) in enumerate(order):
                    f = (G if t == 's' else 0) + (k - 1)
                    pk = P[(t, k)]
                    for c in range(4):
                        nc.tensor.matmul(
                            ps[:, 512 * c:512 * (c + 1)],
                            lhsT=Ws[:, f, :],
                            rhs=pk[:, 512 * c:512 * (c + 1)],
                            start=(mi == 0), stop=(mi == len(order) - 1))
                if relu:
                    nc.vector.tensor_scalar(out=Yout[:, cs], in0=ps[:, :],
                                            scalar1=bias_col, scalar2=0.0,
                                            op0=A.add, op1=A.max)
                else:
                    nc.vector.tensor_scalar_add(out=Yout[:, cs], in0=ps[:, :],
                                                scalar1=bias_col)

        def _pipeline():
            for b in range(BPC):
                tb = b * NT  # token-tile base (in 128-token tiles)

                # ================= LN1 (token-major) =================
                XT = xpool.tile([128, NT, D], F32, tag="xtok")
                nc.sync.dma_start(out=XT[:, :, :], in_=x_r[:, tb:tb + NT, :])
                MV = small.tile([128, NT, 2], F32, tag="mv1")
                ST6 = small.tile([128, 6], F32, tag="st6")
                for i in range(NT):
                    nc.vector.bn_stats(out=ST6[:, :], in_=XT[:, i, :])
                    nc.vector.bn_aggr(out=MV[:, i, :], in_=ST6[:, :])
                R1 = _newton_rsqrt(nc, small, MV[:, :, 1], 128, NT, "n1")

                XN1 = big.tile([128, L], F32, tag="big")
                for q in range(NT // 4):  # 4 transposes per psum bank
                    pt = tpps.tile([128, 512], F32, tag="mm")
                    for j in range(4):
                        i = 4 * q + j
                        xn_t = xnorm.tile([128, D], F32, tag="xn")
                        nc.vector.tensor_scalar(out=xn_t[:, :], in0=XT[:, i, :],
                                                scalar1=MV[:, i, 0:1], scalar2=R1[:, i:i + 1],
                                                op0=A.subtract, op1=A.mult)
                        nc.tensor.transpose(pt[:, 128 * j:128 * (j + 1)], xn_t[:, :], IDN[:, :])
                    nc.vector.tensor_copy(out=XN1[:, 512 * q:512 * (q + 1)], in_=pt[:, :])

                # ================= FKAN1 =================
                Y1 = big.tile([128, L], F32, tag="big")
                fkan(XN1, SC1, SB1, W1s, B1c[:, 0:1], True, Y1)

                # ================= LN2 (dim-major) =================
                Y1SQ = big.tile([128, L], F32, tag="big")
                S_bc = big.tile([128, L], F32, tag="big")
                Q_bc = big.tile([128, L], F32, tag="big")
                for hh in range(2):
                    hs2 = slice(2048 * hh, 2048 * (hh + 1))
                    nc.gpsimd.tensor_tensor(out=Y1SQ[:, hs2], in0=Y1[:, hs2],
                                            in1=Y1[:, hs2], op=A.mult)
                    nc.gpsimd.partition_all_reduce(S_bc[:, hs2], Y1[:, hs2],
                                                   channels=128,
                                                   reduce_op=bass_isa.ReduceOp.add)
                    nc.gpsimd.partition_all_reduce(Q_bc[:, hs2], Y1SQ[:, hs2],
                                                   channels=128,
                                                   reduce_op=bass_isa.ReduceOp.add)
                Srs = small.tile([128, 32], F32, tag="srs")
                nc.sync.dma_start(out=Srs[:, :], in_=S_bc[0:1, :])
                Qrs = small.tile([128, 32], F32, tag="qrs")
                nc.sync.dma_start(out=Qrs[:, :], in_=Q_bc[0:1, :])
                M2 = small.tile([128, 32], F32, tag="m2")
                nc.vector.tensor_scalar_mul(out=M2[:, :], in0=Srs[:, :], scalar1=1.0 / 128)
                T2 = small.tile([128, 32], F32, tag="t2")
                nc.vector.tensor_tensor(out=T2[:, :], in0=M2[:, :], in1=M2[:, :], op=A.mult)
                V2 = small.tile([128, 32], F32, tag="v2")
                nc.vector.scalar_tensor_tensor(out=V2[:, :], in0=Qrs[:, :], scalar=1.0 / 128,
                                               in1=T2[:, :], op0=A.mult, op1=A.subtract)
                R2 = _newton_rsqrt(nc, small, V2[:, :], 128, 32, "n2")
                nc.sync.dma_start(out=rb_d[b, :], in_=R2[:, :])
                R_bc = big.tile([128, L], F32, tag="big")
                nc.sync.dma_start(out=R_bc[:, :], in_=bass.AP(tensor=rb_d, offset=b * L,
                                                              ap=[[0, 128], [1, L]]))
                TC1 = big.tile([128, L], F32, tag="big")
                XN2 = big.tile([128, L], F32, tag="big")
                for hh in range(2):
                    hs2 = slice(2048 * hh, 2048 * (hh + 1))
                    nc.vector.scalar_tensor_tensor(out=TC1[:, hs2], in0=S_bc[:, hs2],
                                                   scalar=-1.0 / 128, in1=Y1[:, hs2],
                                                   op0=A.mult, op1=A.add)
                    nc.vector.tensor_tensor(out=XN2[:, hs2], in0=TC1[:, hs2],
                                            in1=R_bc[:, hs2], op=A.mult)

                # ================= FKAN2 =================
                OUT2 = big.tile([128, L], F32, tag="big")
                fkan(XN2, SC2, SB2, W2s, B2c[:, 0:1], False, OUT2)

                # ================= CBAM channel attention =================
                o3 = OUT2[:, :].rearrange("p (a c) -> p a c", c=32)   # [128, 128blk, 32]
                Bs = small.tile([128, 128], F32, tag="bs")
                nc.vector.tensor_reduce(out=Bs[:, :], in_=o3, axis=mybir.AxisListType.X,
                                        op=A.add)
                Bm = small.tile([128, 128], F32, tag="bm")
                nc.vector.tensor_reduce(out=Bm[:, :], in_=o3, axis=mybir.AxisListType.X,
                                        op=A.max)
                s2 = small.tile([128, 2], F32, tag="s2")
                pcs = typs.tile([128, 512], F32, tag="mm")
                nc.tensor.matmul(pcs[:, 0:1], lhsT=Bs[:, :], rhs=ONESC[:, :],
                                 start=True, stop=True)
                nc.vector.tensor_scalar_mul(out=s2[:, 0:1], in0=pcs[:, 0:1],
                                            scalar1=1.0 / L)
                PMX = small.tile([128, 128], F32, tag="pmx")
                nc.gpsimd.partition_all_reduce(PMX[:, :], Bm[:, :], channels=128,
                                               reduce_op=bass_isa.ReduceOp.max)
                nc.sync.dma_start(out=s2[:, 1:2], in_=PMX[0:1, :])
                ph = typs.tile([128, 512], F32, tag="mm")
                nc.tensor.matmul(ph[0:RED, 0:2], lhsT=W1T[:, :], rhs=s2[:, :],
                                 start=True, stop=True)
                hs = small.tile([RED, 2], F32, tag="hs")
                nc.vector.tensor_scalar_max(out=hs[:, :], in0=ph[0:RED, 0:2], scalar1=0.0)
                pz = typs.tile([128, 512], F32, tag="mm")
                nc.tensor.matmul(pz[:, 0:2], lhsT=W2T[:, :], rhs=hs[:, :],
                                 start=True, stop=True)
                zc = small.tile([128, 2], F32, tag="zc")
                nc.vector.tensor_copy(out=zc[:, :], in_=pz[:, 0:2])
                us = small.tile([128, 1], F32, tag="us")
                nc.vector.tensor_tensor(out=us[:, :], in0=zc[:, 0:1], in1=zc[:, 1:2],
                                        op=A.add)
                th = small.tile([128, 1], F32, tag="th")
                nc.scalar.activation(th[:, :], us[:, :], AF.Tanh, bias=0.0, scale=0.5)
                ca_col = small.tile([128, 1], F32, tag="cac")
                nc.vector.tensor_scalar(out=ca_col[:, :], in0=th[:, :], scalar1=0.5,
                                        scalar2=0.5, op0=A.mult, op1=A.add)
                nc.sync.dma_start(out=cab_d[b, :], in_=ca_col[:, :])
                CA = small.tile([128, 128], F32, tag="cab")
                nc.sync.dma_start(out=CA[:, :], in_=bass.AP(tensor=cab_d, offset=b * D,
                                                            ap=[[0, 128], [1, 128]]))

                X4 = big.tile([128, L], F32, tag="big")
                ca_view = CA[:, :].unsqueeze(2).to_broadcast((128, 128, 32))
                nc.gpsimd.tensor_tensor(out=X4[:, :].rearrange("p (a c) -> p a c", c=32),
                                        in0=o3, in1=ca_view, op=A.mult)
                # note: o3 references OUT2; X4 = OUT2 * ca

                # ================= CBAM spatial attention =================
                x4s = X4[:, :].rearrange("p (a c) -> p c a", c=32)  # [128, 32j, 128c'] strided
                Sms = small.tile([128, 32], F32, tag="sms")
                nc.vector.tensor_reduce(out=Sms[:, :], in_=x4s, axis=mybir.AxisListType.X,
                                        op=A.add)
                Smm = small.tile([128, 32], F32, tag="smm")
                nc.vector.tensor_reduce(out=Smm[:, :], in_=x4s, axis=mybir.AxisListType.X,
                                        op=A.max)
                # transpose (128,32) -> (32,128)
                pts = tpps.tile([128, 512], F32, tag="mm")
                nc.tensor.transpose(pts[0:32, 0:128], Sms[:, :], IDN[:, :])
                nc.tensor.transpose(pts[0:32, 128:256], Smm[:, :], IDN[:, :])
                SmsT = small.tile([32, 134], F32, tag="smst")
                SmmT = small.tile([32, 134], F32, tag="smmt")
                nc.vector.memset(SmsT[:, :], 0.0)
                nc.vector.memset(SmmT[:, :], 0.0)
                nc.vector.tensor_copy(out=SmsT[:, 3:131], in_=pts[0:32, 0:128])
                nc.vector.tensor_copy(out=SmmT[:, 3:131], in_=pts[0:32, 128:256])
                # halos across rows (partition-shifted) via small DMAs
                nc.sync.dma_start(out=SmsT[1:32, 0:3], in_=SmsT[0:31, 125:128])
                nc.sync.dma_start(out=SmsT[0:31, 131:134], in_=SmsT[1:32, 3 + 0:3 + 3])
                nc.sync.dma_start(out=SmmT[1:32, 0:3], in_=SmmT[0:31, 125:128])
                nc.sync.dma_start(out=SmmT[0:31, 131:134], in_=SmmT[1:32, 3 + 0:3 + 3])
                # 7+7 conv taps, ping-pong accumulate
                acc_a = small.tile([32, 128], F32, tag="acca")
                acc_b = small.tile([32, 128], F32, tag="accb")
                nc.vector.tensor_scalar_mul(out=acc_a[:, :], in0=SmsT[:, 0:128],
                                            scalar1=CW[:, 0:1])
                cur, nxt = acc_a, acc_b
                for u in range(1, 7):
                    nc.vector.scalar_tensor_tensor(out=nxt[:, :], in0=SmsT[:, u:u + 128],
                                                   scalar=CW[:, u:u + 1], in1=cur[:, :],
                                                   op0=A.mult, op1=A.add)
                    cur, nxt = nxt, cur
                for u in range(0, 7):
                    nc.vector.scalar_tensor_tensor(out=nxt[:, :], in0=SmmT[:, u:u + 128],
                                                   scalar=CW[:, 7 + u:8 + u], in1=cur[:, :],
                                                   op0=A.mult, op1=A.add)
                    cur, nxt = nxt, cur
                th2 = small.tile([32, 128], F32, tag="th2")
                nc.scalar.activation(th2[:, :], cur[:, :], AF.Tanh, bias=0.0, scale=0.5)
                sas = small.tile([32, 128], F32, tag="sas")
                nc.vector.tensor_scalar(out=sas[:, :], in0=th2[:, :], scalar1=0.5,
                                        scalar2=0.5, op0=A.mult, op1=A.add)
                ptb = tpps.tile([128, 512], F32, tag="mm")
                nc.tensor.transpose(ptb[:, 0:32], sas[:, :], IDN[0:32, 0:32])
                SA = small.tile([128, 32], F32, tag="sab")
                nc.vector.tensor_copy(out=SA[:, :], in_=ptb[:, 0:32])

                # gate + residual + transpose out
                Gt = big.tile([128, L], F32, tag="big")
                sa_view = SA[:, :].unsqueeze(1).to_broadcast((128, 128, 32))
                nc.gpsimd.tensor_tensor(out=Gt[:, :].rearrange("p (a c) -> p a c", c=32),
                                        in0=X4[:, :].rearrange("p (a c) -> p a c", c=32),
                                        in1=sa_view, op=A.mult)
                for q in range(NT // 4):
                    po = tpps.tile([128, 512], F32, tag="mm")
                    for j in range(4):
                        i = 4 * q + j
                        nc.tensor.transpose(po[:, 128 * j:128 * (j + 1)],
                                            Gt[:, 128 * i:128 * (i + 1)], IDN[:, :])
                    ot = otok.tile([128, 4, D], F32, tag="ot")
                    nc.vector.tensor_tensor(out=ot[:, :, :].rearrange("p a d -> p (a d)"),
                                            in0=po[:, :],
                                            in1=XT[:, 4 * q:4 * q + 4, :].rearrange("p a d -> p (a d)"),
                                            op=A.add)
                    nc.sync.dma_start(out=out_r[:, tb + 4 * q:tb + 4 * q + 4, :],
                                      in_=ot[:, :, :])


        if reps == 1:
            _pipeline()
        else:
            with tc.For_i(0, reps, 1):
                _pipeline()

    nc.compile()
    return nc


# ---------------------------------------------------------------- host side
_NC_CACHE = None


def _get_nc():
    global _NC_CACHE
    if _NC_CACHE is None:
        _NC_CACHE = build_program()
    return _NC_CACHE


def _prepare_maps(inputs):
    x = np.ascontiguousarray(np.asarray(inputs["x"], dtype=np.float32))
    fk1_c = np.asarray(inputs["fk1_c"], dtype=np.float32)
    fk2_c = np.asarray(inputs["fk2_c"], dtype=np.float32)
    n1_g = np.asarray(inputs["n1_g"], dtype=np.float32)
    n1_b = np.asarray(inputs["n1_b"], dtype=np.float32)
    n2_g = np.asarray(inputs["n2_g"], dtype=np.float32)
    n2_b = np.asarray(inputs["n2_b"], dtype=np.float32)
    fk1_b = np.asarray(inputs["fk1_b"], dtype=np.float32)
    fk2_b = np.asarray(inputs["fk2_b"], dtype=np.float32)
    w1 = np.asarray(inputs["w1"], dtype=np.float32)
    w2 = np.asarray(inputs["w2"], dtype=np.float32)
    conv_w = np.asarray(inputs["conv_w"], dtype=np.float32)

    # FKAN weights: W[f=t*8+g, i, o] = fk_c[t, o, i, g]
    W1 = np.ascontiguousarray(fk1_c.transpose(0, 3, 2, 1).reshape(NF, D, D)).astype(
        np.float16)
    W2 = np.ascontiguousarray(fk2_c.transpose(0, 3, 2, 1).reshape(NF, D, D)).astype(
        np.float16)

    ks = np.arange(1, G + 1, dtype=np.float64)
    # f = t*8 + (g-1); t=0 -> cos, t=1 -> sin.
    # sc: k*gamma/(2pi) turns-scale for FRAC0.
    # sb: sin cols = k*beta (radians, ACT Sin bias);
    #     cos cols = 0.25 + k*beta/(2pi) (turns, WRAP_HI shift).
    def sc_sb(gam, bet):
        sc = np.empty((D, NF), np.float32)
        sb = np.empty((D, NF), np.float32)
        for t in range(2):
            for gi in range(G):
                f = t * G + gi
                sc[:, f] = (ks[gi] * gam / (2 * np.pi)).astype(np.float32)
                if t == 1:
                    sb[:, f] = (ks[gi] * bet).astype(np.float32)
                else:
                    sb[:, f] = (0.25 + ks[gi] * bet / (2 * np.pi)).astype(np.float32)
        return sc, sb

    sc1, sb1 = sc_sb(n1_g.astype(np.float64), n1_b.astype(np.float64))
    sc2, sb2 = sc_sb(n2_g.astype(np.float64), n2_b.astype(np.float64))

    cw = np.concatenate([conv_w[0, 0, 3, :] / 128.0, conv_w[0, 1, 3, :]]).reshape(1, 14)

    shared = {
        "w1f": W1, "w2f": W2,
        "sc1": sc1, "sb1": sb1, "sc2": sc2, "sb2": sb2,
        "fb1": fk1_b.reshape(D, 1), "fb2": fk2_b.reshape(D, 1),
        "w1t": np.ascontiguousarray(w1.T), "w2t": np.ascontiguousarray(w2.T),
        "cw": cw.astype(np.float32),
    }
    in_maps = []
    for c in range(NCORES):
        m = dict(shared)
        m["x"] = np.ascontiguousarray(x[c * BPC:(c + 1) * BPC].reshape(TOK, D))
        in_maps.append(m)
    return in_maps


def run_raw(inputs, trace=False, **kw):
    nc = _get_nc()
    in_maps = _prepare_maps(inputs)
    res = run_bass_kernel_spmd(nc, in_maps, core_ids=list(range(NCORES)),
                               trace=trace, **kw)
    out = np.stack([res.results[i]["out"].reshape(BPC, L, D) for i in range(NCORES)])
    return out.reshape(B, L, D), res


def kernel(**inputs):
    out, _ = run_raw(inputs, trace=False)
    return out



# revision 20
# speedup vs baseline: 1.6643x; 1.6643x over previous
"""AttFKANBlock Trainium2 Bass kernel (8 NeuronCores, data-parallel over batch).

Pipeline per batch (4096 tokens, dim=128):
  LN1 (token-major, bn_stats + Newton rsqrt + fused apply)
  -> PE transpose to dim-major
  -> FKAN1: FRAC_SCALE custom DVE op (range reduction, folds LN gamma/beta)
            + ACT Sin -> bf16 features -> PE matmul (2048-dim contraction)
  -> LN2 (dim-major, gpsimd partition reduces + broadcasts)
  -> FKAN2
  -> CBAM channel+spatial attention (the torch .view maps channel c'=token//32,
     spatial l' = 128*(token%32)+dim)
  -> residual add in token-major after PE transpose back -> DMA out.
"""
import numpy as np
import ml_dtypes

import concourse.bass as bass
import concourse.bacc as bacc
import concourse.mybir as mybir
import concourse.tile as tile
from concourse import bass_isa
from concourse.bass_utils import run_bass_kernel_spmd

# ---------------------------------------------------------------- custom DVE ops
# FRAC0:   out = u - round_ne(u), u = in0*s0         (4 stages, 2x_2p capable)
# WRAP_HI: out = y - (y > s1),    y = in0 + s0       (3 stages, 2x_2p capable)
from concourse.dve_ops import DveOp, OPS, CUSTOM_DVE_SPECS, _SUB_OPCODE_FOR_NAME
import concourse.dve_ops as _dve_ops_mod
from concourse.dve_spec import Spec, Src0, C0, C1, lower as _dve_lower
from concourse.dve_uop import (DveOpSpec, UopConfig, UopDpConfig, InpSel, OutSel,
                               OutPath, AluInp, DelayInp, AluOp as UAluOp, Trigger,
                               ENABLE, DISABLE)

_MAGIC = 12582912.0  # 1.5 * 2**23


def _ref_frac0(in0, in1, s0, s1, imm2):
    u = np.float32(in0.astype(np.float32) * np.float32(s0))
    v = np.float32(u + np.float32(s1))
    r = np.float32(v - np.float32(s1))
    return np.float32(u - r)


def _ref_wrap_hi(in0, in1, s0, s1, imm2):
    y = np.float32(in0.astype(np.float32) + np.float32(s0))
    return np.float32(y - (y > np.float32(s1)).astype(np.float32))


def _dp(op=UAluOp.BYPASS, a0=AluInp.PREV_ALU_OUT, a1=AluInp.PREV_ALU_OUT,
        dly=None, den=None):
    d = [DelayInp.PREV_DELAY] * 7
    e = [0] * 7
    if dly:
        for k, v in dly.items():
            d[k] = v
    if den:
        for k in den:
            e[k] = 1
    return UopDpConfig(op=op, alu_src0=a0, alu_src1=a1, delay=d,
                       alu_out_enable=1, delay_enable=e)


def _mk_uop_2x2p(blocks):
    u = UopConfig()
    u.inp = [InpSel.ZERO, InpSel.SRC_0, InpSel.CONST_0, InpSel.CONST_1,
             InpSel.SRC_1, InpSel.ZERO, InpSel.ZERO, InpSel.ZERO]
    u.inp_enable = [0, 1, 1, 1, 1, 0, 0, 0]
    u.out = {OutPath.WR0_LO: OutSel.DELAY_0, OutPath.WR0_HI: OutSel.ALU_OUT,
             OutPath.WR1_LO: OutSel.ALU_OUT, OutPath.WR1_HI: OutSel.ALU_OUT}
    u.out_enable = {OutPath.WR0_LO: 1, OutPath.WR0_HI: 1,
                    OutPath.WR1_LO: 0, OutPath.WR1_HI: 0}
    u.trigger = (Trigger.SRC_TENSOR_DONE, Trigger.NONE, Trigger.NONE)
    u.datapath_config = blocks
    return u


PA = AluInp.PREV_ALU_OUT
D0, D1, D2, D3 = (AluInp.PREV_DELAY_0, AluInp.PREV_DELAY_1,
                  AluInp.PREV_DELAY_2, AluInp.PREV_DELAY_3)
DP_PASS = DelayInp.PREV_DELAY
DP_ALU = DelayInp.PREV_ALU_OUT
DP_CUR = DelayInp.CURR_ALU_OUT

# FRAC0 2x_2p: chain A on b0-b3 (in SRC_0 -> WR0 via delay0), chain B on b4-b7
_FRAC0_2X2P = _mk_uop_2x2p([
    # b0: uA = srcA * C0 ; pass C0(d1) C1(d2) srcB(d3)
    _dp(UAluOp.MULTIPLY, D0, D1, den=[1, 2, 3]),
    # b1: uA + C1 ; capture uA on d0; pass others
    _dp(UAluOp.ADD, PA, D2, dly={0: DP_ALU}, den=[0, 1, 2, 3]),
    # b2: rA = (uA+C1) - C1 ; pass uA(d0) C0(d1) C1(d2) srcB(d3)
    _dp(UAluOp.SUBTRACT, PA, D2, den=[0, 1, 2, 3]),
    # b3: outA = uA - rA ; capture outA on d0; pass C0 C1 srcB
    _dp(UAluOp.SUBTRACT, D0, PA, dly={0: DP_CUR}, den=[0, 1, 2, 3]),
    # b4: uB = srcB * C0 ; pass outA(d0) C1(d2)
    _dp(UAluOp.MULTIPLY, D3, D1, den=[0, 2]),
    # b5: uB + C1 ; capture uB on d1; pass outA(d0) C1(d2)
    _dp(UAluOp.ADD, PA, D2, dly={1: DP_ALU}, den=[0, 1, 2]),
    # b6: rB ; pass outA(d0) uB(d1)
    _dp(UAluOp.SUBTRACT, PA, D2, den=[0, 1]),
    # b7: outB = uB - rB ; pass outA(d0)
    _dp(UAluOp.SUBTRACT, D1, PA, den=[0]),
])

# WRAP_HI 2x_2p: chain A b0-b2, chain B b3-b5, bypass b6-b7
_WRAP_2X2P = _mk_uop_2x2p([
    # b0: yA = srcA + C0 ; pass C0(d1) C1(d2) srcB(d3)
    _dp(UAluOp.ADD, D0, D1, den=[1, 2, 3]),
    # b1: gA = yA > C1 ; capture yA on d0
    _dp(UAluOp.IS_GT, PA, D2, dly={0: DP_ALU}, den=[0, 1, 2, 3]),
    # b2: outA = yA - gA ; capture outA on d0
    _dp(UAluOp.SUBTRACT, D0, PA, dly={0: DP_CUR}, den=[0, 1, 2, 3]),
    # b3: yB = srcB + C0 ; pass outA(d0) C1(d2)
    _dp(UAluOp.ADD, D3, D1, den=[0, 2]),
    # b4: gB = yB > C1 ; capture yB on d1
    _dp(UAluOp.IS_GT, PA, D2, dly={1: DP_ALU}, den=[0, 1, 2]),
    # b5: outB = yB - gB ; pass outA(d0)
    _dp(UAluOp.SUBTRACT, D1, PA, den=[0]),
    # b6, b7: bypass outB down the ALU path, outA down d0
    _dp(UAluOp.BYPASS, PA, PA, den=[0]),
    _dp(UAluOp.BYPASS, PA, PA, den=[0]),
])


def _register_op(name, spec, uops_2x2p):
    if name in _SUB_OPCODE_FOR_NAME:
        return next(op for op in OPS if op.name == name)
    row = max(_SUB_OPCODE_FOR_NAME.values()) + 1
    assert row < 0x20
    _SUB_OPCODE_FOR_NAME[name] = row
    shas = {}
    specs = {}
    for ver in ("v3", "v4"):
        u1 = _dve_lower(spec, ver=ver)
        import copy as _copy
        ds = DveOpSpec(name=name, opcode=row, uops=u1,
                       uops_2x=[_copy.deepcopy(x) for x in u1],
                       uops_2x_2p=[uops_2x2p] if ver == "v3" else None,
                       uops_4x=None,
                       perf_max=2 if ver == "v3" else 0,
                       rd1_en=False)
        if ver != "v3":
            ds = DveOpSpec(name=name, opcode=row, uops=u1, rd1_en=False)
        shas[ver] = ds.sha(ver)
        specs[ver] = ds
    op = DveOp(name, spec, subdim=False, uops_sha=shas)
    OPS.append(op)
    CUSTOM_DVE_SPECS[name] = spec
    for ver in ("v3", "v4"):
        _dve_ops_mod._COMPILE_CACHE[(name, ver)] = specs[ver]
    return op


_u0 = Src0 * C0
FRAC0 = _register_op("FRAC0_ANT",
                     Spec(body=_u0 - ((_u0 + C1) - C1), reference=_ref_frac0),
                     _FRAC0_2X2P)
_y0 = Src0 + C0
WRAP_HI = _register_op("WRAP_HI_ANT",
                       Spec(body=_y0 - (_y0 > C1), reference=_ref_wrap_hi),
                       _WRAP_2X2P)

from concourse.dve_spec import C2 as _C2


def _ref_frac_ph(in0, in1, s0, s1, imm2):
    u = np.float32(in0.astype(np.float32) * np.float32(s0) + np.float32(s1))
    v = np.float32(u + np.float32(imm2))
    r = np.float32(v - np.float32(imm2))
    return np.float32(u - r)


def _register_plain(name, spec):
    if name in _SUB_OPCODE_FOR_NAME:
        return next(op for op in OPS if op.name == name)
    row = max(_SUB_OPCODE_FOR_NAME.values()) + 1
    assert row < 0x20
    _SUB_OPCODE_FOR_NAME[name] = row
    shas = {}
    for ver in ("v3", "v4"):
        ds = DveOpSpec(name=name, opcode=row, uops=_dve_lower(spec, ver=ver),
                       rd1_en=False)
        shas[ver] = ds.sha(ver)
    op = DveOp(name, spec, subdim=False, uops_sha=shas)
    OPS.append(op)
    CUSTOM_DVE_SPECS[name] = spec
    return op


_uph = Src0 * C0 + C1
FRAC_PH = _register_plain("FRAC_PH_ANT",
                          Spec(body=_uph - ((_uph + _C2) - _C2),
                               reference=_ref_frac_ph))


def _frac_ph(nc, out, in_, s0, s1):
    return nc.vector._custom_dve(FRAC_PH, out=out, in0=in_, s0=s0, s1=s1,
                                 imm2=_MAGIC)


from concourse.dve_ops import get_dve_sub_opcode as _row_of


def _custom_dve_pm(nc, op, out, in0, in1=None, s0=0.0, s1=0.0, imm2=0.0,
                   perf_max=0):
    """_custom_dve clone that encodes perf_max in byte-36[7:6] so the engine
    can reach the registered 2x/2x_2p uop slots (bass default leaves it 0 =
    REGULAR-only)."""
    v = nc.vector
    if op.name not in nc.m.ant_custom_dve_ops:
        nc.m.ant_custom_dve_ops = sorted({*nc.m.ant_custom_dve_ops, op.name})
    compiled = op.compile("v3")
    ins = [v.lower_ap(in0, for_isa=True, opt=True)]
    if in1 is not None:
        ins.append(v.lower_ap(in1, for_isa=True, opt=True))

    def _sc(x):
        if isinstance(x, (int, float)):
            return mybir.ImmediateValue(dtype=mybir.dt.float32, value=float(x))
        return v.lower_ap(x, for_isa=True)

    ins += [_sc(s0), _sc(s1)]
    shape = bass_isa.CustomDveShape.TTSS
    return v.add_instruction(bass_isa.InstCustomDveAnt(
        name=nc.get_next_instruction_name(),
        op_name=op.name,
        rd1_en=compiled.rd1_en,
        subdim=0,
        imm2=imm2,
        shape=shape,
        row=_row_of(op.name),
        isa_opcode=nc.isa.Opcode[
            f"NEURON_ISA_TPB_OPCODE_CUSTOM_DVE_ANT_{shape.slot()}"].value,
        ins=ins,
        outs=[v.lower_ap(out, for_isa=True, opt=True)],
        perf_max=perf_max,
    ))


# ---- harmonic-ladder ops (2-input, f16, 2x_1p packed: 2 tokens/cycle) ----
# LMAD: out = (in0*s0)*in1 - s1   (c2k = 2*ck*ck - 1, s2k = 2*ck*sk)
# LODD: out = (in0*s0 + s1)*in1   (c3 = (2c2-1)*c1, s3 = (2c2+1)*s1)
from concourse.dve_spec import Src1 as _Src1


def _ref_lmad(in0, in1, s0, s1, imm2):
    return np.float32(np.float32(in0.astype(np.float32) * np.float32(s0))
                      * in1.astype(np.float32) - np.float32(s1))


def _ref_lodd(in0, in1, s0, s1, imm2):
    return np.float32(np.float32(in0.astype(np.float32) * np.float32(s0)
                                 + np.float32(s1)) * in1.astype(np.float32))


def _mk_uop_tt2x(blocks):
    """TT-shaped 2x_1p uop: lanes a=SRC_0, b=SRC_1, C0, C1, a'=SRC_0_HI,
    b'=SRC_1_HI; chain A on b0-b2 -> delay0, chain B on b3-b5 -> ALU out."""
    u = UopConfig()
    u.inp = [InpSel.ZERO, InpSel.SRC_0, InpSel.SRC_1, InpSel.CONST_0,
             InpSel.CONST_1, InpSel.SRC_0_HI, InpSel.SRC_1_HI, InpSel.ZERO]
    u.inp_enable = [0, 1, 1, 1, 1, 1, 1, 0]
    u.out = {OutPath.WR0_LO: OutSel.DELAY_0, OutPath.WR0_HI: OutSel.ALU_OUT,
             OutPath.WR1_LO: OutSel.ALU_OUT, OutPath.WR1_HI: OutSel.ALU_OUT}
    u.out_enable = {OutPath.WR0_LO: 1, OutPath.WR0_HI: 1,
                    OutPath.WR1_LO: 0, OutPath.WR1_HI: 0}
    u.trigger = (Trigger.SRC_TENSOR_DONE, Trigger.NONE, Trigger.NONE)
    u.datapath_config = blocks
    return u


D4, D5 = AluInp.PREV_DELAY_4, AluInp.PREV_DELAY_5

# lane map at block0: D0=a, D1=b, D2=C0, D3=C1, D4=a_hi, D5=b_hi
_LMAD_2X = _mk_uop_tt2x([
    _dp(UAluOp.MULTIPLY, D0, D2, den=[1, 2, 3, 4, 5]),      # tA = a*C0
    _dp(UAluOp.MULTIPLY, PA, D1, den=[2, 3, 4, 5]),          # tA *= b
    _dp(UAluOp.SUBTRACT, PA, D3, dly={0: DP_CUR},            # outA = tA - C1
        den=[0, 2, 3, 4, 5]),
    _dp(UAluOp.MULTIPLY, D4, D2, den=[0, 3, 5]),             # tB = a'*C0
    _dp(UAluOp.MULTIPLY, PA, D5, den=[0, 3]),                # tB *= b'
    _dp(UAluOp.SUBTRACT, PA, D3, den=[0]),                   # outB = tB - C1
    _dp(UAluOp.BYPASS, PA, PA, den=[0]),
    _dp(UAluOp.BYPASS, PA, PA, den=[0]),
])

_LODD_2X = _mk_uop_tt2x([
    _dp(UAluOp.MULTIPLY, D0, D2, den=[1, 2, 3, 4, 5]),      # tA = a*C0
    _dp(UAluOp.ADD, PA, D3, den=[1, 2, 3, 4, 5]),            # tA += C1
    _dp(UAluOp.MULTIPLY, PA, D1, dly={0: DP_CUR},            # outA = tA*b
        den=[0, 2, 3, 4, 5]),
    _dp(UAluOp.MULTIPLY, D4, D2, den=[0, 3, 5]),             # tB = a'*C0
    _dp(UAluOp.ADD, PA, D3, den=[0, 5]),                     # tB += C1
    _dp(UAluOp.MULTIPLY, PA, D5, den=[0]),                   # outB = tB*b'
    _dp(UAluOp.BYPASS, PA, PA, den=[0]),
    _dp(UAluOp.BYPASS, PA, PA, den=[0]),
])


def _register_tt_op(name, spec, uops_2x1p):
    if name in _SUB_OPCODE_FOR_NAME:
        return next(op for op in OPS if op.name == name)
    row = max(_SUB_OPCODE_FOR_NAME.values()) + 1
    assert row < 0x20
    _SUB_OPCODE_FOR_NAME[name] = row
    shas = {}
    specs = {}
    for ver in ("v3", "v4"):
        u1 = _dve_lower(spec, ver=ver)
        if ver == "v3":
            assert len(u1) == 1, f"{name}: expected single-uop lowering"
            ds = DveOpSpec(name=name, opcode=row, uops=u1,
                           uops_2x=[uops_2x1p], uops_2x_2p=None, uops_4x=None,
                           perf_max=1, rd1_en=True)
        else:
            ds = DveOpSpec(name=name, opcode=row, uops=u1, rd1_en=True)
        shas[ver] = ds.sha(ver)
        specs[ver] = ds
    op = DveOp(name, spec, subdim=False, uops_sha=shas)
    OPS.append(op)
    CUSTOM_DVE_SPECS[name] = spec
    for ver in ("v3", "v4"):
        _dve_ops_mod._COMPILE_CACHE[(name, ver)] = specs[ver]
    return op


LMAD = _register_tt_op("LMAD_ANT",
                       Spec(body=(Src0 * C0) * _Src1 - C1, reference=_ref_lmad),
                       _LMAD_2X)
LODD = _register_tt_op("LODD_ANT",
                       Spec(body=(Src0 * C0 + C1) * _Src1, reference=_ref_lodd),
                       _LODD_2X)


def _lmad(nc, out, a, b, s0, s1):
    return _custom_dve_pm(nc, LMAD, out, a, in1=b, s0=s0, s1=s1,
                          perf_max=PM_LADDER)


def _lodd(nc, out, a, b, s0, s1):
    return _custom_dve_pm(nc, LODD, out, a, in1=b, s0=s0, s1=s1,
                          perf_max=PM_LADDER)


import os
EPI_ACT = int(os.environ.get("EPI_ACT", "0"))


def _frac0(nc, out, in_, s0):
    return nc.vector._custom_dve(FRAC0, out=out, in0=in_, s0=s0, s1=_MAGIC)


def _wrap_hi(nc, out, in_, s0):
    return nc.vector._custom_dve(WRAP_HI, out=out, in0=in_, s0=s0, s1=0.5)


# ---------------------------------------------------------------- constants
B, L, D, G = 16, 4096, 128, 8
RED = 8          # D // 16
NF = 2 * G       # 16 features per input dim (cos/sin x 8 harmonics)
NCORES = 8
BPC = B // NCORES          # 2 batches per core
TOK = BPC * L              # 8192 tokens per core
PI = float(np.pi)
EPS = 1e-5
NT = L // 128              # 32 token tiles per batch
A = mybir.AluOpType
F32, BF16, F16 = mybir.dt.float32, mybir.dt.bfloat16, mybir.dt.float16
AF = mybir.ActivationFunctionType


def _newton_rsqrt(nc, pool, var_ap, p, n, tag):
    """rsqrt(var + EPS) on a [p, n] f32 tile chain. Returns R tile [p, n]."""
    vp = pool.tile([p, n], F32, tag=f"{tag}_v")
    nc.vector.tensor_scalar_add(out=vp[:, :], in0=var_ap, scalar1=EPS)
    y = pool.tile([p, n], F32, tag=f"{tag}_y")
    nc.vector.tensor_scalar(out=y[:, :], in0=vp[:, :], scalar1=-0.5, scalar2=1.5,
                            op0=A.mult, op1=A.add)
    nc.vector.tensor_scalar_max(out=y[:, :], in0=y[:, :], scalar1=0.19)
    a_t = pool.tile([p, n], F32, tag=f"{tag}_a")
    c_t = pool.tile([p, n], F32, tag=f"{tag}_c")
    for _ in range(6):
        nc.vector.tensor_tensor(out=a_t[:, :], in0=y[:, :], in1=y[:, :], op=A.mult)
        nc.vector.scalar_tensor_tensor(out=c_t[:, :], in0=vp[:, :], scalar=-0.5,
                                       in1=a_t[:, :], op0=A.mult, op1=A.mult)
        nc.vector.scalar_tensor_tensor(out=y[:, :], in0=c_t[:, :], scalar=1.5,
                                       in1=y[:, :], op0=A.add, op1=A.mult)
    return y


def build_program(reps=1):
    nc = bacc.Bacc("TRN2", target_bir_lowering=False, debug=False, num_devices=NCORES,
                   enable_asserts=False)
    x_d = nc.dram_tensor("x", [TOK, D], F32, kind="ExternalInput")
    w1_d = nc.dram_tensor("w1f", [NF, D, D], F16, kind="ExternalInput")
    w2_d = nc.dram_tensor("w2f", [NF, D, D], F16, kind="ExternalInput")
    sc1_d = nc.dram_tensor("sc1", [D, NF], F32, kind="ExternalInput")
    sb1_d = nc.dram_tensor("sb1", [D, NF], F32, kind="ExternalInput")
    sc2_d = nc.dram_tensor("sc2", [D, NF], F32, kind="ExternalInput")
    sb2_d = nc.dram_tensor("sb2", [D, NF], F32, kind="ExternalInput")
    b1_d = nc.dram_tensor("fb1", [D, 1], F32, kind="ExternalInput")
    b2_d = nc.dram_tensor("fb2", [D, 1], F32, kind="ExternalInput")
    w1t_d = nc.dram_tensor("w1t", [D, RED], F32, kind="ExternalInput")
    w2t_d = nc.dram_tensor("w2t", [RED, D], F32, kind="ExternalInput")
    cw_d = nc.dram_tensor("cw", [1, 14], F32, kind="ExternalInput")
    out_d = nc.dram_tensor("out", [TOK, D], F32, kind="ExternalOutput")
    rb_d = nc.dram_tensor("rbounce", [BPC, L], F32)
    cab_d = nc.dram_tensor("cabounce", [BPC, D], F32)

    from contextlib import ExitStack
    from concourse.masks import make_identity

    with tile.TileContext(nc) as tc, ExitStack() as ctx:
        singles = ctx.enter_context(tc.tile_pool(name="singles", bufs=1))
        xpool = ctx.enter_context(tc.tile_pool(name="xtok", bufs=2))
        big = ctx.enter_context(tc.tile_pool(name="big", bufs=4))
        mpool = ctx.enter_context(tc.tile_pool(name="mtile", bufs=2))
        fpool = ctx.enter_context(tc.tile_pool(name="ftile", bufs=1))
        small = ctx.enter_context(tc.tile_pool(name="small", bufs=2))
        rpool = ctx.enter_context(tc.tile_pool(name="rrow", bufs=1))
        xnorm = ctx.enter_context(tc.tile_pool(name="xnorm", bufs=6))
        otok = ctx.enter_context(tc.tile_pool(name="otok", bufs=3))
        mmps = ctx.enter_context(tc.tile_pool(name="mmps", bufs=2, space="PSUM"))
        tpps = mmps
        typs = mmps

        # ---- constants / weights resident in SBUF
        W1s = singles.tile([D, NF, D], F16)
        nc.sync.dma_start(out=W1s[:, :, :], in_=w1_d.ap().rearrange("f i o -> i f o"))
        W2s = singles.tile([D, NF, D], F16)
        nc.sync.dma_start(out=W2s[:, :, :], in_=w2_d.ap().rearrange("f i o -> i f o"))
        SC1 = singles.tile([D, NF], F32)
        nc.sync.dma_start(out=SC1[:, :], in_=sc1_d[:, :])
        SB1 = singles.tile([D, NF], F32)
        nc.sync.dma_start(out=SB1[:, :], in_=sb1_d[:, :])
        SC2 = singles.tile([D, NF], F32)
        nc.sync.dma_start(out=SC2[:, :], in_=sc2_d[:, :])
        SB2 = singles.tile([D, NF], F32)
        nc.sync.dma_start(out=SB2[:, :], in_=sb2_d[:, :])
        B1c = singles.tile([D, 1], F32)
        nc.sync.dma_start(out=B1c[:, :], in_=b1_d[:, :])
        B2c = singles.tile([D, 1], F32)
        nc.sync.dma_start(out=B2c[:, :], in_=b2_d[:, :])
        W1T = singles.tile([D, RED], F32)
        nc.sync.dma_start(out=W1T[:, :], in_=w1t_d[:, :])
        W2T = singles.tile([RED, D], F32)
        nc.sync.dma_start(out=W2T[:, :], in_=w2t_d[:, :])
        CW = singles.tile([32, 14], F32)
        nc.sync.dma_start(out=CW[:, :], in_=bass.AP(tensor=cw_d, offset=0,
                                                    ap=[[0, 32], [1, 14]]))
        IDN = singles.tile([D, D], F32)
        make_identity(nc, IDN[:, :])
        ONESC = singles.tile([D, 1], F32)
        nc.vector.memset(ONESC[:, :], 1.0)
        ONES128 = singles.tile([D, D], F32)
        nc.vector.memset(ONES128[:, :], 1.0)

        x_r = x_d.ap().rearrange("(a p) d -> p a d", p=128)      # [128, 64, 128]
        out_r = out_d.ap().rearrange("(a p) d -> p a d", p=128)  # [128, 64, 128]

        def fkan(XN, SC, SB, Ws, bias_col, relu, Yout):
            """XN (128 dims x 4096 tok f32) -> Yout (128 out x 4096 tok f32).

            Bases c1,s1 via FRAC+ACT Sin; 14 product-basis planes via
            built-in TT mult (f16 2x_1p). Host folds the harmonic->product
            change of basis into the weights (see _prepare_maps)."""
            TT = nc.vector.tensor_tensor
            for half in range(2):
                cs = slice(2048 * half, 2048 * (half + 1))
                ps = mmps.tile([128, 2048], F32, tag="mm")
                fb = mpool.tile([128, 2048], F16, tag="m")
                _frac0(nc, fb[:, :], XN[:, cs], SC[:, G:G + 1])
                s1 = fpool.tile([128, 2048], F16, tag="s1")
                nc.scalar.activation(s1[:, :], fb[:, :], AF.Sin,
                                     bias=SB[:, G:G + 1], scale=2 * PI)
                fb2 = mpool.tile([128, 2048], F16, tag="m")
                _frac_ph(nc, fb2[:, :], XN[:, cs], SC[:, 0:1], SB[:, 0:1])
                c1 = fpool.tile([128, 2048], F16, tag="c1")
                nc.scalar.activation(c1[:, :], fb2[:, :], AF.Sin,
                                     bias=0.0, scale=2 * PI)
                P = {0: c1, 1: s1}

                def mk(idx, a, b):
                    t = fpool.tile([128, 2048], F16, tag=f"p{idx}")
                    TT(out=t[:, :], in0=a[:, :], in1=b[:, :], op=A.mult)
                    P[idx] = t
                    return t

                # planes 2..15: P2,Q2,P3,Q3,P4,Q4,P5,Q5,P6,Q6,P7,Q7,P8,Q8
                P2 = mk(2, c1, c1)
                Q2 = mk(3, c1, s1)
                R2 = fpool.tile([128, 2048], F16, tag="r2")
                nc.vector.tensor_scalar_add(out=R2[:, :], in0=P2[:, :],
                                            scalar1=-1.0)
                P3 = mk(4, R2, c1)
                Q3 = mk(5, R2, s1)
                P4 = mk(6, R2, P2)
                Q4 = mk(7, Q2, P2)
                mk(8, P4, c1)
                mk(9, P4, s1)
                mk(10, P3, P3)
                mk(11, P3, Q3)
                mk(12, P4, P3)
                mk(13, P4, Q3)
                mk(14, P4, P4)
                mk(15, P4, Q4)
                for mi in range(16):
                    for c in range(4):
                        nc.tensor.matmul(
                            ps[:, 512 * c:512 * (c + 1)],
                            lhsT=Ws[:, mi, :],
                            rhs=P[mi][:, 512 * c:512 * (c + 1)],
                            start=(mi == 0), stop=(mi == 15))
                if EPI_ACT:
                    nc.scalar.activation(Yout[:, cs], ps[:, :],
                                         AF.Relu if relu else AF.Identity,
                                         bias=bias_col, scale=1.0)
                elif relu:
                    nc.vector.tensor_scalar(out=Yout[:, cs], in0=ps[:, :],
                                            scalar1=bias_col, scalar2=0.0,
                                            op0=A.add, op1=A.max)
                else:
                    nc.vector.tensor_scalar_add(out=Yout[:, cs], in0=ps[:, :],
                                                scalar1=bias_col)

        def _pipeline():
            for b in range(BPC):
                tb = b * NT  # token-tile base (in 128-token tiles)

                # ================= LN1 (token-major) =================
                XT = xpool.tile([128, NT, D], F32, tag="xtok")
                nc.sync.dma_start(out=XT[:, :, :], in_=x_r[:, tb:tb + NT, :])
                MV = small.tile([128, NT, 2], F32, tag="mv1")
                ST6 = small.tile([128, 6], F32, tag="st6")
                for i in range(NT):
                    nc.vector.bn_stats(out=ST6[:, :], in_=XT[:, i, :])
                    nc.vector.bn_aggr(out=MV[:, i, :], in_=ST6[:, :])
                R1 = _newton_rsqrt(nc, small, MV[:, :, 1], 128, NT, "n1")

                XN1 = big.tile([128, L], F32, tag="big")
                for q in range(NT // 4):  # 4 transposes per psum bank
                    pt = tpps.tile([128, 512], F32, tag="mm")
                    for j in range(4):
                        i = 4 * q + j
                        xn_t = xnorm.tile([128, D], F32, tag="xn")
                        nc.vector.tensor_scalar(out=xn_t[:, :], in0=XT[:, i, :],
                                                scalar1=MV[:, i, 0:1], scalar2=R1[:, i:i + 1],
                                                op0=A.subtract, op1=A.mult)
                        nc.tensor.transpose(pt[:, 128 * j:128 * (j + 1)], xn_t[:, :], IDN[:, :])
                    nc.vector.tensor_copy(out=XN1[:, 512 * q:512 * (q + 1)], in_=pt[:, :])

                # ================= FKAN1 =================
                Y1 = big.tile([128, L], F32, tag="big")
                fkan(XN1, SC1, SB1, W1s, B1c[:, 0:1], True, Y1)

                # ================= LN2 (dim-major) =================
                Y1SQ = big.tile([128, L], F32, tag="big")
                S_bc = big.tile([128, L], F32, tag="big")
                Q_bc = big.tile([128, L], F32, tag="big")
                for hh in range(2):
                    hs2 = slice(2048 * hh, 2048 * (hh + 1))
                    nc.scalar.activation(Y1SQ[:, hs2], Y1[:, hs2], AF.Square,
                                         bias=0.0, scale=1.0)
                # per-token sums over the 128 dims via ones-matmul (output is
                # already broadcast across partitions); replaces gpsimd
                # partition_all_reduce on the critical path
                for hh in range(2):
                    for c in range(4):
                        col = slice(2048 * hh + 512 * c, 2048 * hh + 512 * (c + 1))
                        sp = mmps.tile([128, 512], F32, tag="mm")
                        nc.tensor.matmul(sp[:, :], lhsT=ONES128[:, :],
                                         rhs=Y1[:, col], start=True, stop=True)
                        nc.vector.tensor_copy(out=S_bc[:, col], in_=sp[:, :])
                        qp = mmps.tile([128, 512], F32, tag="mm")
                        nc.tensor.matmul(qp[:, :], lhsT=ONES128[:, :],
                                         rhs=Y1SQ[:, col], start=True, stop=True)
                        nc.vector.tensor_copy(out=Q_bc[:, col], in_=qp[:, :])
                Srs = small.tile([128, 32], F32, tag="srs")
                nc.sync.dma_start(out=Srs[:, :], in_=S_bc[0:1, :])
                Qrs = small.tile([128, 32], F32, tag="qrs")
                nc.sync.dma_start(out=Qrs[:, :], in_=Q_bc[0:1, :])
                M2 = small.tile([128, 32], F32, tag="m2")
                nc.vector.tensor_scalar_mul(out=M2[:, :], in0=Srs[:, :], scalar1=1.0 / 128)
                T2 = small.tile([128, 32], F32, tag="t2")
                nc.vector.tensor_tensor(out=T2[:, :], in0=M2[:, :], in1=M2[:, :], op=A.mult)
                V2 = small.tile([128, 32], F32, tag="v2")
                nc.vector.scalar_tensor_tensor(out=V2[:, :], in0=Qrs[:, :], scalar=1.0 / 128,
                                               in1=T2[:, :], op0=A.mult, op1=A.subtract)
                R2 = _newton_rsqrt(nc, small, V2[:, :], 128, 32, "n2")
                nc.sync.dma_start(out=rb_d[b, :], in_=R2[:, :])
                R_bc = big.tile([128, L], F32, tag="big")
                nc.sync.dma_start(out=R_bc[:, :], in_=bass.AP(tensor=rb_d, offset=b * L,
                                                              ap=[[0, 128], [1, L]]))
                TC1 = big.tile([128, L], F32, tag="big")
                XN2 = big.tile([128, L], F32, tag="big")
                for hh in range(2):
                    hs2 = slice(2048 * hh, 2048 * (hh + 1))
                    nc.vector.scalar_tensor_tensor(out=TC1[:, hs2], in0=S_bc[:, hs2],
                                                   scalar=-1.0 / 128, in1=Y1[:, hs2],
                                                   op0=A.mult, op1=A.add)
                    nc.vector.tensor_tensor(out=XN2[:, hs2], in0=TC1[:, hs2],
                                            in1=R_bc[:, hs2], op=A.mult)

                # ================= FKAN2 =================
                OUT2 = big.tile([128, L], F32, tag="big")
                fkan(XN2, SC2, SB2, W2s, B2c[:, 0:1], False, OUT2)

                # ================= CBAM channel attention =================
                o3 = OUT2[:, :].rearrange("p (a c) -> p a c", c=32)   # [128, 128blk, 32]
                Bs = small.tile([128, 128], F32, tag="bs")
                nc.vector.tensor_reduce(out=Bs[:, :], in_=o3, axis=mybir.AxisListType.X,
                                        op=A.add)
                Bm = small.tile([128, 128], F32, tag="bm")
                nc.vector.tensor_reduce(out=Bm[:, :], in_=o3, axis=mybir.AxisListType.X,
                                        op=A.max)
                s2 = small.tile([128, 2], F32, tag="s2")
                pcs = typs.tile([128, 512], F32, tag="mm")
                nc.tensor.matmul(pcs[:, 0:1], lhsT=Bs[:, :], rhs=ONESC[:, :],
                                 start=True, stop=True)
                nc.vector.tensor_scalar_mul(out=s2[:, 0:1], in0=pcs[:, 0:1],
                                            scalar1=1.0 / L)
                PMX = small.tile([128, 128], F32, tag="pmx")
                nc.gpsimd.partition_all_reduce(PMX[:, :], Bm[:, :], channels=128,
                                               reduce_op=bass_isa.ReduceOp.max)
                nc.sync.dma_start(out=s2[:, 1:2], in_=PMX[0:1, :])
                ph = typs.tile([128, 512], F32, tag="mm")
                nc.tensor.matmul(ph[0:RED, 0:2], lhsT=W1T[:, :], rhs=s2[:, :],
                                 start=True, stop=True)
                hs = small.tile([RED, 2], F32, tag="hs")
                nc.vector.tensor_scalar_max(out=hs[:, :], in0=ph[0:RED, 0:2], scalar1=0.0)
                pz = typs.tile([128, 512], F32, tag="mm")
                nc.tensor.matmul(pz[:, 0:2], lhsT=W2T[:, :], rhs=hs[:, :],
                                 start=True, stop=True)
                zc = small.tile([128, 2], F32, tag="zc")
                nc.vector.tensor_copy(out=zc[:, :], in_=pz[:, 0:2])
                us = small.tile([128, 1], F32, tag="us")
                nc.vector.tensor_tensor(out=us[:, :], in0=zc[:, 0:1], in1=zc[:, 1:2],
                                        op=A.add)
                th = small.tile([128, 1], F32, tag="th")
                nc.scalar.activation(th[:, :], us[:, :], AF.Tanh, bias=0.0, scale=0.5)
                ca_col = small.tile([128, 1], F32, tag="cac")
                nc.vector.tensor_scalar(out=ca_col[:, :], in0=th[:, :], scalar1=0.5,
                                        scalar2=0.5, op0=A.mult, op1=A.add)
                nc.sync.dma_start(out=cab_d[b, :], in_=ca_col[:, :])
                CA = small.tile([128, 128], F32, tag="cab")
                nc.sync.dma_start(out=CA[:, :], in_=bass.AP(tensor=cab_d, offset=b * D,
                                                            ap=[[0, 128], [1, 128]]))

                X4 = big.tile([128, L], F32, tag="big")
                ca_view = CA[:, :].unsqueeze(2).to_broadcast((128, 128, 32))
                nc.gpsimd.tensor_tensor(out=X4[:, :].rearrange("p (a c) -> p a c", c=32),
                                        in0=o3, in1=ca_view, op=A.mult)
                # note: o3 references OUT2; X4 = OUT2 * ca

                # ================= CBAM spatial attention =================
                x4s = X4[:, :].rearrange("p (a c) -> p c a", c=32)  # [128, 32j, 128c'] strided
                Sms = small.tile([128, 32], F32, tag="sms")
                nc.vector.tensor_reduce(out=Sms[:, :], in_=x4s, axis=mybir.AxisListType.X,
                                        op=A.add)
                Smm = small.tile([128, 32], F32, tag="smm")
                nc.vector.tensor_reduce(out=Smm[:, :], in_=x4s, axis=mybir.AxisListType.X,
                                        op=A.max)
                # transpose (128,32) -> (32,128)
                pts = tpps.tile([128, 512], F32, tag="mm")
                nc.tensor.transpose(pts[0:32, 0:128], Sms[:, :], IDN[:, :])
                nc.tensor.transpose(pts[0:32, 128:256], Smm[:, :], IDN[:, :])
                SmsT = small.tile([32, 134], F32, tag="smst")
                SmmT = small.tile([32, 134], F32, tag="smmt")
                nc.vector.memset(SmsT[:, :], 0.0)
                nc.vector.memset(SmmT[:, :], 0.0)
                nc.vector.tensor_copy(out=SmsT[:, 3:131], in_=pts[0:32, 0:128])
                nc.vector.tensor_copy(out=SmmT[:, 3:131], in_=pts[0:32, 128:256])
                # halos across rows (partition-shifted) via small DMAs
                nc.sync.dma_start(out=SmsT[1:32, 0:3], in_=SmsT[0:31, 125:128])
                nc.sync.dma_start(out=SmsT[0:31, 131:134], in_=SmsT[1:32, 3 + 0:3 + 3])
                nc.sync.dma_start(out=SmmT[1:32, 0:3], in_=SmmT[0:31, 125:128])
                nc.sync.dma_start(out=SmmT[0:31, 131:134], in_=SmmT[1:32, 3 + 0:3 + 3])
                # 7+7 conv taps, ping-pong accumulate
                acc_a = small.tile([32, 128], F32, tag="acca")
                acc_b = small.tile([32, 128], F32, tag="accb")
                nc.vector.tensor_scalar_mul(out=acc_a[:, :], in0=SmsT[:, 0:128],
                                            scalar1=CW[:, 0:1])
                cur, nxt = acc_a, acc_b
                for u in range(1, 7):
                    nc.vector.scalar_tensor_tensor(out=nxt[:, :], in0=SmsT[:, u:u + 128],
                                                   scalar=CW[:, u:u + 1], in1=cur[:, :],
                                                   op0=A.mult, op1=A.add)
                    cur, nxt = nxt, cur
                for u in range(0, 7):
                    nc.vector.scalar_tensor_tensor(out=nxt[:, :], in0=SmmT[:, u:u + 128],
                                                   scalar=CW[:, 7 + u:8 + u], in1=cur[:, :],
                                                   op0=A.mult, op1=A.add)
                    cur, nxt = nxt, cur
                th2 = small.tile([32, 128], F32, tag="th2")
                nc.scalar.activation(th2[:, :], cur[:, :], AF.Tanh, bias=0.0, scale=0.5)
                sas = small.tile([32, 128], F32, tag="sas")
                nc.vector.tensor_scalar(out=sas[:, :], in0=th2[:, :], scalar1=0.5,
                                        scalar2=0.5, op0=A.mult, op1=A.add)
                ptb = tpps.tile([128, 512], F32, tag="mm")
                nc.tensor.transpose(ptb[:, 0:32], sas[:, :], IDN[0:32, 0:32])
                SA = small.tile([128, 32], F32, tag="sab")
                nc.vector.tensor_copy(out=SA[:, :], in_=ptb[:, 0:32])

                # gate + residual + transpose out
                Gt = big.tile([128, L], F32, tag="big")
                sa_view = SA[:, :].unsqueeze(1).to_broadcast((128, 128, 32))
                nc.gpsimd.tensor_tensor(out=Gt[:, :].rearrange("p (a c) -> p a c", c=32),
                                        in0=X4[:, :].rearrange("p (a c) -> p a c", c=32),
                                        in1=sa_view, op=A.mult)
                for q in range(NT // 4):
                    po = tpps.tile([128, 512], F32, tag="mm")
                    for j in range(4):
                        i = 4 * q + j
                        nc.tensor.transpose(po[:, 128 * j:128 * (j + 1)],
                                            Gt[:, 128 * i:128 * (i + 1)], IDN[:, :])
                    ot = otok.tile([128, 4, D], F32, tag="ot")
                    nc.vector.tensor_tensor(out=ot[:, :, :].rearrange("p a d -> p (a d)"),
                                            in0=po[:, :],
                                            in1=XT[:, 4 * q:4 * q + 4, :].rearrange("p a d -> p (a d)"),
                                            op=A.add)
                    nc.sync.dma_start(out=out_r[:, tb + 4 * q:tb + 4 * q + 4, :],
                                      in_=ot[:, :, :])


        if reps == 1:
            _pipeline()
        else:
            with tc.For_i(0, reps, 1):
                _pipeline()

    nc.compile()
    return nc


# ---------------------------------------------------------------- host side
_COEF_CACHE = None


def _basis_coef():
    """[17, 16] map: harmonic feature f (cos k=1..8 then sin k=1..8) =
    sum_g coef[g,f]*plane_g + coef[16,f]. Planes follow fkan's order."""
    global _COEF_CACHE
    if _COEF_CACHE is not None:
        return _COEF_CACHE
    th = np.linspace(0, 2 * np.pi, 64, endpoint=False)
    c1, s1 = np.cos(th), np.sin(th)
    P2, Q2 = c1 * c1, c1 * s1
    R2 = P2 - 1.0
    P3, Q3 = R2 * c1, R2 * s1
    P4, Q4 = R2 * P2, Q2 * P2
    H = np.stack([c1, s1, P2, Q2, P3, Q3, P4, Q4, P4 * c1, P4 * s1,
                  P3 * P3, P3 * Q3, P4 * P3, P4 * Q3, P4 * P4, P4 * Q4,
                  np.ones_like(th)], 1)
    ks = np.arange(1, G + 1)
    F = np.concatenate([np.cos(np.outer(th, ks)), np.sin(np.outer(th, ks))], 1)
    coef, _, _, _ = np.linalg.lstsq(H, F, rcond=None)
    assert np.abs(H @ coef - F).max() < 1e-9
    _COEF_CACHE = coef
    return coef


_NC_CACHE = None


def _get_nc():
    global _NC_CACHE
    if _NC_CACHE is None:
        _NC_CACHE = build_program()
    return _NC_CACHE


def _prepare_maps(inputs):
    x = np.ascontiguousarray(np.asarray(inputs["x"], dtype=np.float32))
    fk1_c = np.asarray(inputs["fk1_c"], dtype=np.float32)
    fk2_c = np.asarray(inputs["fk2_c"], dtype=np.float32)
    n1_g = np.asarray(inputs["n1_g"], dtype=np.float32)
    n1_b = np.asarray(inputs["n1_b"], dtype=np.float32)
    n2_g = np.asarray(inputs["n2_g"], dtype=np.float32)
    n2_b = np.asarray(inputs["n2_b"], dtype=np.float32)
    fk1_b = np.asarray(inputs["fk1_b"], dtype=np.float32)
    fk2_b = np.asarray(inputs["fk2_b"], dtype=np.float32)
    w1 = np.asarray(inputs["w1"], dtype=np.float32)
    w2 = np.asarray(inputs["w2"], dtype=np.float32)
    conv_w = np.asarray(inputs["conv_w"], dtype=np.float32)

    # FKAN weights: W[f=t*8+g, i, o] = fk_c[t, o, i, g]
    W1h = fk1_c.transpose(0, 3, 2, 1).reshape(NF, D, D).astype(np.float64)
    W2h = fk2_c.transpose(0, 3, 2, 1).reshape(NF, D, D).astype(np.float64)
    # change of basis: harmonics -> product planes (kernel computes
    # [c1,s1,c1c1,c1s1,R2c1,R2s1,R2P2,Q2P2,P4c1,P4s1,P3P3,P3Q3,P4P3,P4Q3,
    #  P4P4,P4Q4] with R2=P2-1). coef[g,f]: F_f = sum_g coef[g]*H_g + coef[16].
    coef = _basis_coef()
    W1p = np.einsum("gf,fio->gio", coef[:16], W1h)
    W2p = np.einsum("gf,fio->gio", coef[:16], W2h)
    b1p = np.asarray(fk1_b, np.float64) + np.einsum("f,fio->o", coef[16], W1h)
    b2p = np.asarray(fk2_b, np.float64) + np.einsum("f,fio->o", coef[16], W2h)
    W1 = np.ascontiguousarray(W1p).astype(np.float16)
    W2 = np.ascontiguousarray(W2p).astype(np.float16)

    ks = np.arange(1, G + 1, dtype=np.float64)
    # f = t*8 + (g-1); t=0 -> cos, t=1 -> sin.
    # sc: k*gamma/(2pi) turns-scale for FRAC0.
    # sb: sin cols = k*beta (radians, ACT Sin bias);
    #     cos cols = 0.25 + k*beta/(2pi) (turns, WRAP_HI shift).
    def sc_sb(gam, bet):
        sc = np.empty((D, NF), np.float32)
        sb = np.empty((D, NF), np.float32)
        for t in range(2):
            for gi in range(G):
                f = t * G + gi
                sc[:, f] = (ks[gi] * gam / (2 * np.pi)).astype(np.float32)
                if t == 1:
                    sb[:, f] = (ks[gi] * bet).astype(np.float32)
                else:
                    sb[:, f] = (0.25 + ks[gi] * bet / (2 * np.pi)).astype(np.float32)
        return sc, sb

    sc1, sb1 = sc_sb(n1_g.astype(np.float64), n1_b.astype(np.float64))
    sc2, sb2 = sc_sb(n2_g.astype(np.float64), n2_b.astype(np.float64))

    cw = np.concatenate([conv_w[0, 0, 3, :] / 128.0, conv_w[0, 1, 3, :]]).reshape(1, 14)

    shared = {
        "w1f": W1, "w2f": W2,
        "sc1": sc1, "sb1": sb1, "sc2": sc2, "sb2": sb2,
        "fb1": b1p.astype(np.float32).reshape(D, 1),
        "fb2": b2p.astype(np.float32).reshape(D, 1),
        "w1t": np.ascontiguousarray(w1.T), "w2t": np.ascontiguousarray(w2.T),
        "cw": cw.astype(np.float32),
    }
    in_maps = []
    for c in range(NCORES):
        m = dict(shared)
        m["x"] = np.ascontiguousarray(x[c * BPC:(c + 1) * BPC].reshape(TOK, D))
        in_maps.append(m)
    return in_maps


def run_raw(inputs, trace=False, **kw):
    nc = _get_nc()
    in_maps = _prepare_maps(inputs)
    res = run_bass_kernel_spmd(nc, in_maps, core_ids=list(range(NCORES)),
                               trace=trace, **kw)
    out = np.stack([res.results[i]["out"].reshape(BPC, L, D) for i in range(NCORES)])
    return out.reshape(B, L, D), res


def kernel(**inputs):
    out, _ = run_raw(inputs, trace=False)
    return out

